# revision 28
# baseline (speedup 1.0000x reference)
"""Trainium2 Bass kernel for nn_CSRN_76922864272117.

Quad-directional conv-GRU spatial RNN (CSRN). B=8,C=256,H=W=64.
Reference computes 4 directional scans but the 'left' scan result is dead
(overwritten) and ctx_right is zeros, so only 3 scans (down, up, right)
and the first 3 C-blocks of w_comb contribute to the output.

Sharding: data-parallel over batch, 1 sample per NeuronCore (8 cores).
Per core: 3 independent 64-step recurrent chains, gates computed in
[g-tile, l] layout (128 partitions), gi+gh fused in PSUM, conv1d as 3
shifted matmuls (+K=1 bias matmul), sigmoid/tanh on ScalarE from PSUM,
final 768->256 combination matmul tail. All matmul inputs bf16 (fp32
PSUM accumulate): measured rel_err vs fp32 reference ~5e-3.

Host-side prep (outside the NEFF): transpose/cast weights & x into the
SBUF-friendly layouts; gather = np.stack of per-core outputs.
"""
import sys
if '/opt/trn_rl_repo' not in sys.path:
    sys.path.insert(0, '/opt/trn_rl_repo')

import numpy as np
import ml_dtypes

B, C, H, W = 8, 256, 64, 64
T = 64          # scan steps (H or W)
L = 64          # line length per step
NCORES = 8
BF16 = ml_dtypes.bfloat16

_CACHE = {}
DEBUG_CTX = False
FOLD_CB = False   # emit K=1 conv-bias matmuls (needed only if cb != 0)
ABLATE = frozenset()   # timing-only ablations: gi, gh, conv, dve, comb


def _build(nsteps):
    import concourse.bass as bass
    import concourse.tile as tile
    import concourse.mybir as mybir
    from concourse import bacc

    dt = mybir.dt
    AF = mybir.ActivationFunctionType
    nc = bacc.Bacc(None, target_bir_lowering=False)

    DIRS = ('d', 'u', 'r')
    # ---- DRAM parameters (host-prepped layouts) ----
    xhw = nc.dram_tensor("xhw", [128, 2, H, W], dt.bfloat16, kind="ExternalInput")
    xwh = nc.dram_tensor("xwh", [128, 2, W, H], dt.bfloat16, kind="ExternalInput")
    wih, whh, cwT, cb = {}, {}, {}, {}
    for X in DIRS:
        wih[X] = nc.dram_tensor(f"wihT_{X}", [128, 2, 768], dt.bfloat16, kind="ExternalInput")
        whh[X] = nc.dram_tensor(f"whhT_{X}", [128, 2, 768], dt.bfloat16, kind="ExternalInput")
        cwT[X] = nc.dram_tensor(f"cwT_{X}", [128, 2, 3, 256], dt.bfloat16, kind="ExternalInput")
        cb[X] = nc.dram_tensor(f"cb_{X}", [1, 256], dt.bfloat16, kind="ExternalInput")
    wcombT = nc.dram_tensor("wcombT", [128, 6, 256], dt.bfloat16, kind="ExternalInput")
    bcomb = nc.dram_tensor("bcomb", [1, 256], dt.bfloat16, kind="ExternalInput")
    out_d = nc.dram_tensor("out", [256, H, W], dt.float32, kind="ExternalOutput")
    dbg = {}
    if DEBUG_CTX:
        for X in DIRS:
            dbg[X] = nc.dram_tensor(f"dbgctx_{X}", [128, 2, T, L], dt.bfloat16,
                                    kind="ExternalOutput")

    with tile.TileContext(nc) as tc:
        with (
            tc.tile_pool(name="big", bufs=1) as big,
            tc.tile_pool(name="work", bufs=3) as work,
            tc.tile_pool(name="ob", bufs=3) as obp,
            tc.tile_pool(name="ps", bufs=2, space="PSUM") as ps,
        ):
            # ---- persistent SBUF tensors ----
            x_sb = {}
            x_sb['d'] = big.tile([128, 2, H, W], dt.bfloat16, tag="xhw", name="xhw_sb")
            x_sb['u'] = x_sb['d']
            x_sb['r'] = big.tile([128, 2, W, H], dt.bfloat16, tag="xwh", name="xwh_sb")
            nc.sync.dma_start(x_sb['d'][:], xhw[:])
            nc.sync.dma_start(x_sb['r'][:], xwh[:])
            wih_sb, whh_sb, cw_sb, cb_sb, h_sb, ctx_sb = {}, {}, {}, {}, {}, {}
            for X in DIRS:
                wih_sb[X] = big.tile([128, 2, 768], dt.bfloat16, tag=f"wih{X}", name=f"wih{X}_sb")
                whh_sb[X] = big.tile([128, 2, 768], dt.bfloat16, tag=f"whh{X}", name=f"whh{X}_sb")
                cw_sb[X] = big.tile([128, 2, 3, 256], dt.bfloat16, tag=f"cw{X}", name=f"cw{X}_sb")
                cb_sb[X] = big.tile([1, 256], dt.bfloat16, tag=f"cb{X}", name=f"cb{X}_sb")
                nc.sync.dma_start(wih_sb[X][:], wih[X][:])
                nc.sync.dma_start(whh_sb[X][:], whh[X][:])
                nc.sync.dma_start(cw_sb[X][:], cwT[X][:])
                nc.sync.dma_start(cb_sb[X][:], cb[X][:])
                h_sb[X] = big.tile([128, 2, L], dt.bfloat16, tag=f"h{X}", name=f"h{X}_sb")
                nc.vector.memset(h_sb[X][:], 0.0)
                ctx_sb[X] = big.tile([128, 2, T, L], dt.bfloat16, tag=f"ctx{X}", name=f"ctx{X}_sb")
                if nsteps < T:
                    nc.vector.memset(ctx_sb[X][:], 0.0)  # debug-only builds
            wc_sb = big.tile([128, 6, 256], dt.bfloat16, tag="wcomb")
            bc_sb = big.tile([1, 256], dt.bfloat16, tag="bcomb")
            nc.sync.dma_start(wc_sb[:], wcombT[:])
            nc.sync.dma_start(bc_sb[:], bcomb[:])
            ones = big.tile([1, 512], dt.bfloat16, tag="ones")
            nc.vector.memset(ones[:], 1.0)

            def xrow(X, t):
                return t if X == 'd' else (T - 1 - t)

            # NOTE: start=True clears has_written for the WHOLE psum bank, so
            # each 512-wide psg tile gets exactly one start (first matmul of
            # the step) and one stop (last); first write to each element
            # overwrites (bit clear), later writes accumulate.
            def emit_gi(X, t, psg):
                xs = x_sb[X]
                row = xrow(X, t)
                for gt in range(6):
                    col = gt * 64 if gt < 4 else 256 + (gt - 4) * 64
                    for kc in range(2):
                        nc.tensor.matmul(
                            psg[:, col:col + 64],
                            wih_sb[X][:, kc, gt * 128:(gt + 1) * 128],
                            xs[:, kc, row, :],
                            start=(gt == 0 and kc == 0), stop=False,
                            skip_group_check=True)

            def emit_gh(X, t, psg):
                h = h_sb[X]
                # n-part (cols 384:512, first write overwrites), then rz accum
                for gt in (4, 5):
                    col = 384 + (gt - 4) * 64
                    for kc in range(2):
                        nc.tensor.matmul(
                            psg[:, col:col + 64],
                            whh_sb[X][:, kc, gt * 128:(gt + 1) * 128],
                            h[:, kc, :],
                            start=False, stop=False, skip_group_check=True)
                for gt in range(4):
                    col = gt * 64
                    for kc in range(2):
                        nc.tensor.matmul(
                            psg[:, col:col + 64],
                            whh_sb[X][:, kc, gt * 128:(gt + 1) * 128],
                            h[:, kc, :],
                            start=False, stop=(gt == 3 and kc == 1),
                            skip_group_check=True)

            def emit_conv(X, t, cy):
                # y[o,l] = sum_k sum_i cw[o,i,k] ctx[i,l+k-1], zero pad
                ctxrow = ctx_sb[X][:, :, xrow(X, t), :]  # [128,2,L]
                first = True
                for ot in range(2):
                    for kc in range(2):
                        for k in range(3):
                            if k == 0:
                                o_sl, i_sl = (1, L), (0, L - 1)
                            elif k == 1:
                                o_sl, i_sl = (0, L), (0, L)
                            else:
                                o_sl, i_sl = (0, L - 1), (1, L)
                            last = (not FOLD_CB and ot == 1 and kc == 1 and k == 2)
                            nc.tensor.matmul(
                                cy[:, ot, o_sl[0]:o_sl[1]],
                                cw_sb[X][:, kc, k, ot * 128:(ot + 1) * 128],
                                ctxrow[:, kc, i_sl[0]:i_sl[1]],
                                start=first, stop=last,
                                skip_group_check=True)
                            first = False
                    if FOLD_CB:
                        nc.tensor.matmul(
                            cy[:, ot, :], cb_sb[X][:, ot * 128:(ot + 1) * 128],
                            ones[:, 0:L], start=False, stop=(ot == 1),
                            skip_group_check=True)

            AFT, AFS = AF.Tanh, AF.Sigmoid
            AB = ABLATE
            psg_t = {}
            cy_t = {}
            for X in DIRS:
                psg_t[X] = ps.tile([128, 512], dt.float32, tag=f"g{X}", name=f"psg{X}")
                if 'gi' not in AB:
                    emit_gi(X, 0, psg_t[X])
            for t in range(nsteps):
                DIRS_T = DIRS
                for X in DIRS_T:
                    if 'gh' not in AB:
                        emit_gh(X, t, psg_t[X])
                rz = {}
                for X in DIRS_T:
                    rz[X] = work.tile([128, 256], dt.bfloat16, tag=f"rz{X}", name=f"rz{X}")
                    nc.scalar.activation(rz[X][:], psg_t[X][:, 0:256], AFS)
                t1, npre = {}, {}
                for X in DIRS_T:
                    if 'dve' in AB:
                        npre[X] = psg_t[X][:, 256:384].rearrange("p (a b) -> p a b", a=2)
                        continue
                    t1[X] = work.tile([128, 2, 64], dt.float32, tag=f"t1{X}", name=f"t1{X}")
                    nc.vector.tensor_mul(
                        t1[X][:], rz[X][:, 0:128].rearrange("p (a b) -> p a b", a=2),
                        psg_t[X][:, 384:512].rearrange("p (a b) -> p a b", a=2))
                    npre[X] = work.tile([128, 2, 64], dt.float32, tag=f"np{X}", name=f"np{X}")
                    nc.vector.tensor_add(
                        npre[X][:], t1[X][:],
                        psg_t[X][:, 256:384].rearrange("p (a b) -> p a b", a=2))
                nn = {}
                for X in DIRS_T:
                    nn[X] = work.tile([128, 2, 64], dt.bfloat16, tag=f"nn{X}", name=f"nn{X}")
                    nc.scalar.activation(nn[X][:], npre[X][:], AFT)
                for X in DIRS_T:
                    if 'dve' in AB:
                        nc.vector.tensor_copy(ctx_sb[X][:, :, xrow(X, t), :], nn[X][:])
                        continue
                    d1 = work.tile([128, 2, 64], dt.bfloat16, tag=f"d1{X}", name=f"d1{X}")
                    nc.vector.tensor_sub(d1[:], h_sb[X][:], nn[X][:])
                    d2 = work.tile([128, 2, 64], dt.bfloat16, tag=f"d2{X}", name=f"d2{X}")
                    nc.vector.tensor_mul(
                        d2[:], rz[X][:, 128:256].rearrange("p (a b) -> p a b", a=2), d1[:])
                    ctxrow = ctx_sb[X][:, :, xrow(X, t), :]
                    nc.vector.tensor_add(ctxrow, nn[X][:], d2[:])
                # next-step gi fills PE while elementwise completes
                if t + 1 < nsteps:
                    nxt = {}
                    for X in DIRS_T:
                        nxt[X] = ps.tile([128, 512], dt.float32, tag=f"g{X}", name=f"psg{X}")
                        if 'gi' not in AB:
                            emit_gi(X, t + 1, nxt[X])
                for X in DIRS_T:
                    if 'conv' in AB:
                        continue
                    cy_t[X] = ps.tile([128, 2, L], dt.float32, tag="convy", name=f"cy{X}")
                    emit_conv(X, t, cy_t[X])
                for X in DIRS_T:
                    src_ap = (psg_t[X][:, 0:128].rearrange("p (a b) -> p a b", a=2)
                              if 'conv' in AB else cy_t[X][:])
                    nc.scalar.activation(h_sb[X][:], src_ap, AFT)
                if t + 1 < nsteps:
                    psg_t = nxt

            if DEBUG_CTX:
                for X in DIRS:
                    nc.sync.dma_start(dbg[X][:], ctx_sb[X][:])
            # ---- combination matmul: out[o, h, w] ----
            for ot in range(2):
                if 'comb' in AB:
                    continue
                for ch in range(8):
                    h0 = ch * 8
                    pc = ps.tile([128, 512], dt.float32, tag="gd", name="pc")
                    first = True
                    for j, X in enumerate(DIRS):
                        for kc in range(2):
                            if X == 'r':
                                rhs = ctx_sb[X][:, kc, :, h0:h0 + 8].rearrange(
                                    "p w h -> p h w")
                            else:
                                rhs = ctx_sb[X][:, kc, h0:h0 + 8, :]
                            nc.tensor.matmul(
                                pc[:], wc_sb[:, j * 2 + kc, ot * 128:(ot + 1) * 128],
                                rhs, start=first, stop=False)
                            first = False
                    nc.tensor.matmul(pc[:], bc_sb[:, ot * 128:(ot + 1) * 128],
                                     ones[:], start=False, stop=True)
                    ob = obp.tile([128, 512], dt.float32, tag="ob")
                    if ch % 2 == 0:
                        nc.scalar.activation(ob[:], pc[:], AF.Copy)
                    else:
                        nc.vector.tensor_copy(ob[:], pc[:])
                    nc.sync.dma_start(
                        out_d[ot * 128:(ot + 1) * 128, h0:h0 + 8, :],
                        ob[:].rearrange("p (a b) -> p a b", a=8))

    nc.compile()
    return nc


def _prep_core_inputs(inputs, b):
    """Host-side layout prep for one batch sample (pure numpy, no device)."""
    def bf(a):
        return np.ascontiguousarray(a).astype(BF16)

    x = np.asarray(inputs['x'][b], np.float32)          # [C, H, W]
    m = {}
    m['xhw'] = bf(x.reshape(2, 128, H, W).transpose(1, 0, 2, 3))
    xw = x.transpose(0, 2, 1)                           # [C, W, H]
    m['xwh'] = bf(xw.reshape(2, 128, W, H).transpose(1, 0, 2, 3))
    for X in ('d', 'u', 'r'):
        wihm = np.asarray(inputs[f'wih_{X}'], np.float32)   # [768, 256]
        whhm = np.asarray(inputs[f'whh_{X}'], np.float32)
        cwm = np.asarray(inputs[f'cw_{X}'], np.float32)     # [256, 256, 3]
        cbm = np.asarray(inputs[f'cb_{X}'], np.float32)
        m[f'wihT_{X}'] = bf(wihm.T.reshape(2, 128, 768).transpose(1, 0, 2))
        m[f'whhT_{X}'] = bf(whhm.T.reshape(2, 128, 768).transpose(1, 0, 2))
        m[f'cwT_{X}'] = bf(cwm.transpose(1, 2, 0).reshape(2, 128, 3, 256)
                           .transpose(1, 0, 2, 3))
        m[f'cb_{X}'] = bf(cbm.reshape(1, 256))
    wc = np.asarray(inputs['w_comb'], np.float32)        # [256, 1024]
    m['wcombT'] = bf(wc[:, :768].T.reshape(3, 2, 128, 256)
                     .transpose(2, 0, 1, 3).reshape(128, 6, 256))
    m['bcomb'] = bf(np.asarray(inputs['b_comb'], np.float32).reshape(1, 256))
    return m


def _get_exec(nsteps=T):
    """Build + compile + wrap in a reusable jitted 8-core executable."""
    key = ('exec', nsteps, FOLD_CB)
    if key in _CACHE:
        return _CACHE[key]
    import jax
    jax.devices()  # initialize the PJRT plugin so platform "neuron" exists
    from jax.sharding import Mesh, PartitionSpec, NamedSharding
    from jax.experimental.shard_map import shard_map
    from concourse import bass2jax
    from jax.interpreters import mlir
    # bass2jax's import-time registration silently skips if the plugin
    # wasn't initialized yet; re-register unconditionally.
    mlir.register_lowering(bass2jax._bass_exec_p,
                           bass2jax._bass_exec_neuron_lowering,
                           platform="neuron")

    nc = _build(nsteps)
    _CACHE[('nc', nsteps)] = nc
    bass2jax.install_neuronx_cc_hook()

    in_names = ['xhw', 'xwh']
    for X in ('d', 'u', 'r'):
        in_names += [f'wihT_{X}', f'whhT_{X}', f'cwT_{X}', f'cb_{X}']
    in_names += ['wcombT', 'bcomb']
    out_names = ['out']
    out_avals = [jax.core.ShapedArray((256, H, W), np.float32)]
    if DEBUG_CTX:
        for X in ('d', 'u', 'r'):
            out_names.append(f'dbgctx_{X}')
            out_avals.append(jax.core.ShapedArray((128, 2, T, L), BF16))
    partition_name = nc.partition_id_tensor.name if nc.partition_id_tensor else None
    all_names = list(in_names) + out_names + ([partition_name] if partition_name else [])

    def _body(*args):
        operands = list(args)
        if partition_name is not None:
            operands.append(bass2jax.partition_id_tensor())
        outs = bass2jax._bass_exec_p.bind(
            *operands,
            out_avals=tuple(out_avals),
            in_names=tuple(all_names),
            out_names=tuple(out_names),
            lowering_input_output_aliases=(),
            sim_require_finite=True,
            sim_require_nnan=True,
            nc=nc,
        )
        return tuple(outs)

    devices = jax.devices()[:NCORES]
    mesh = Mesh(np.asarray(devices), ("core",))
    n_in = len(in_names) + len(out_names)
    sharded = jax.jit(
        shard_map(_body, mesh=mesh, in_specs=(PartitionSpec("core"),) * n_in,
                  out_specs=(PartitionSpec("core"),) * len(out_names),
                  check_rep=False),
        keep_unused=True,
    )
    sharding = NamedSharding(mesh, PartitionSpec("core"))
    _CACHE[key] = (sharded, in_names, sharding)
    return _CACHE[key]


def prep_concat_inputs(inputs, in_names):
    per_core = [_prep_core_inputs(inputs, b) for b in range(B)]
    cat = [np.concatenate([per_core[b][n] for b in range(B)], axis=0)
           for n in in_names]
    cat.append(np.zeros((B * 256, H, W), np.float32))   # donated out buffer
    if DEBUG_CTX:
        for _ in range(3):
            cat.append(np.zeros((B * 128, 2, T, L), BF16))
    return cat


def kernel(**inputs):
    import jax
    global FOLD_CB
    FOLD_CB = any(np.any(np.asarray(inputs[f'cb_{X}'])) for X in ('d', 'u', 'r'))
    sharded, in_names, sharding = _get_exec(T)
    cat = prep_concat_inputs(inputs, in_names)
    cat_d = [jax.device_put(a, sharding) for a in cat]
    (out,) = sharded(*cat_d)
    out = np.asarray(out).reshape(B, 256, H, W)
    return out


if __name__ == '__main__':
    # quick smoke test with random inputs
    rng = np.random.default_rng(0)
    ins = {'x': rng.standard_normal((B, C, H, W), np.float32) * 0.1}
    for X in ('d', 'u', 'l', 'r'):
        ins[f'wih_{X}'] = rng.standard_normal((768, 256), np.float32) / 16
        ins[f'whh_{X}'] = rng.standard_normal((768, 256), np.float32) / 16
        ins[f'cw_{X}'] = rng.standard_normal((256, 256, 3), np.float32) / 16
        ins[f'cb_{X}'] = np.zeros(256, np.float32)
    ins['w_comb'] = rng.standard_normal((256, 1024), np.float32) / 32
    ins['b_comb'] = np.zeros(256, np.float32)
    o = kernel(**ins)
    print("out", o.shape, o.dtype, float(np.abs(o).max()))


# revision 32
# speedup vs baseline: 1.4832x; 1.4832x over previous
"""Trainium2 Bass kernel for nn_CSRN_76922864272117.

Quad-directional conv-GRU spatial RNN (CSRN). B=8,C=256,H=W=64.
Reference computes 4 directional scans but the 'left' scan result is dead
(overwritten) and ctx_right is zeros, so only 3 scans (down, up, right)
and the first 3 C-blocks of w_comb contribute to the output.

Sharding: data-parallel over batch, 1 sample per NeuronCore (8 cores).
Per core: 3 independent 64-step recurrent chains, gates computed in
[g-tile, l] layout (128 partitions), gi+gh fused in PSUM, conv1d as 3
shifted matmuls (+K=1 bias matmul), sigmoid/tanh on ScalarE from PSUM,
final 768->256 combination matmul tail. All matmul inputs bf16 (fp32
PSUM accumulate): measured rel_err vs fp32 reference ~5e-3.

Host-side prep (outside the NEFF): transpose/cast weights & x into the
SBUF-friendly layouts; gather = np.stack of per-core outputs.
"""
import sys
if '/opt/trn_rl_repo' not in sys.path:
    sys.path.insert(0, '/opt/trn_rl_repo')

import numpy as np
import ml_dtypes

B, C, H, W = 8, 256, 64, 64
T = 64          # scan steps (H or W)
L = 64          # line length per step
NCORES = 8
BF16 = ml_dtypes.bfloat16

_CACHE = {}
DEBUG_CTX = False
FOLD_CB = False   # emit K=1 conv-bias matmuls (needed only if cb != 0)
ABLATE = frozenset()   # timing-only ablations: gi, gh, conv, dve, comb


def _build(nsteps):
    import concourse.bass as bass
    import concourse.tile as tile
    import concourse.mybir as mybir
    from concourse import bacc

    dt = mybir.dt
    AF = mybir.ActivationFunctionType
    nc = bacc.Bacc(None, target_bir_lowering=False)

    DIRS = ('d', 'u', 'r')
    # ---- DRAM parameters (host-prepped layouts) ----
    xhw = nc.dram_tensor("xhw", [128, 2, H, W], dt.bfloat16, kind="ExternalInput")
    xwh = nc.dram_tensor("xwh", [128, 2, W, H], dt.bfloat16, kind="ExternalInput")
    wih, whh, cwT, cb = {}, {}, {}, {}
    for X in DIRS:
        wih[X] = nc.dram_tensor(f"wihT_{X}", [128, 2, 768], dt.bfloat16, kind="ExternalInput")
        whh[X] = nc.dram_tensor(f"whhT_{X}", [128, 2, 768], dt.bfloat16, kind="ExternalInput")
        cwT[X] = nc.dram_tensor(f"cwT_{X}", [128, 2, 3, 256], dt.bfloat16, kind="ExternalInput")
        cb[X] = nc.dram_tensor(f"cb_{X}", [1, 256], dt.bfloat16, kind="ExternalInput")
    wcombT = nc.dram_tensor("wcombT", [128, 6, 256], dt.bfloat16, kind="ExternalInput")
    bcomb = nc.dram_tensor("bcomb", [1, 256], dt.bfloat16, kind="ExternalInput")
    out_d = nc.dram_tensor("out", [256, H, W], dt.float32, kind="ExternalOutput")
    dbg = {}
    if DEBUG_CTX:
        for X in DIRS:
            dbg[X] = nc.dram_tensor(f"dbgctx_{X}", [128, 2, T, L], dt.bfloat16,
                                    kind="ExternalOutput")

    with tile.TileContext(nc) as tc:
        with (
            tc.tile_pool(name="big", bufs=1) as big,
            tc.tile_pool(name="work", bufs=3) as work,
            tc.tile_pool(name="ob", bufs=3) as obp,
            tc.tile_pool(name="ps", bufs=2, space="PSUM") as ps,
        ):
            # ---- persistent SBUF tensors ----
            x_sb = {}
            x_sb['d'] = big.tile([128, 2, H, W], dt.bfloat16, tag="xhw", name="xhw_sb")
            x_sb['u'] = x_sb['d']
            x_sb['r'] = big.tile([128, 2, W, H], dt.bfloat16, tag="xwh", name="xwh_sb")
            wih_sb, whh_sb, cw_sb, cb_sb, h_sb, ctx_sb = {}, {}, {}, {}, {}, {}
            for X in DIRS:
                wih_sb[X] = big.tile([128, 2, 768], dt.bfloat16, tag=f"wih{X}", name=f"wih{X}_sb")
                whh_sb[X] = big.tile([128, 2, 768], dt.bfloat16, tag=f"whh{X}", name=f"whh{X}_sb")
                cw_sb[X] = big.tile([128, 2, 3, 256], dt.bfloat16, tag=f"cw{X}", name=f"cw{X}_sb")
                cb_sb[X] = big.tile([1, 256], dt.bfloat16, tag=f"cb{X}", name=f"cb{X}_sb")
                h_sb[X] = big.tile([128, 2, L], dt.bfloat16, tag=f"h{X}", name=f"h{X}_sb")
                nc.vector.memset(h_sb[X][:], 0.0)
                ctx_sb[X] = big.tile([128, 2, T, L], dt.bfloat16, tag=f"ctx{X}", name=f"ctx{X}_sb")
                if nsteps < T:
                    nc.vector.memset(ctx_sb[X][:], 0.0)  # debug-only builds
            # DMA order = first-use order: per-dir gi weights + the x rows
            # each scan touches first, then recurrence weights, then the x
            # remainders in consumption order (d forward, u/r backward).
            nc.sync.dma_start(wih_sb['d'][:], wih['d'][:])
            nc.sync.dma_start(x_sb['d'][:, :, 0:8, :], xhw[:, :, 0:8, :])
            nc.sync.dma_start(wih_sb['u'][:], wih['u'][:])
            nc.sync.dma_start(x_sb['d'][:, :, 56:64, :], xhw[:, :, 56:64, :])
            nc.sync.dma_start(wih_sb['r'][:], wih['r'][:])
            nc.sync.dma_start(x_sb['r'][:, :, 56:64, :], xwh[:, :, 56:64, :])
            for X in DIRS:
                nc.sync.dma_start(whh_sb[X][:], whh[X][:])
                nc.sync.dma_start(cw_sb[X][:], cwT[X][:])
                nc.sync.dma_start(cb_sb[X][:], cb[X][:])
            nc.sync.dma_start(x_sb['d'][:, :, 40:56, :], xhw[:, :, 40:56, :])
            nc.sync.dma_start(x_sb['r'][:, :, 40:56, :], xwh[:, :, 40:56, :])
            nc.sync.dma_start(x_sb['d'][:, :, 8:40, :], xhw[:, :, 8:40, :])
            nc.sync.dma_start(x_sb['r'][:, :, 0:40, :], xwh[:, :, 0:40, :])
            wc_sb = big.tile([128, 6, 256], dt.bfloat16, tag="wcomb")
            bc_sb = big.tile([1, 256], dt.bfloat16, tag="bcomb")
            nc.sync.dma_start(wc_sb[:], wcombT[:])
            nc.sync.dma_start(bc_sb[:], bcomb[:])
            ones = big.tile([1, 512], dt.bfloat16, tag="ones")
            nc.vector.memset(ones[:], 1.0)

            def xrow(X, t):
                return t if X == 'd' else (T - 1 - t)

            # NOTE: start=True clears has_written for the WHOLE psum bank, so
            # each 512-wide psg tile gets exactly one start (first matmul of
            # the step) and one stop (last); first write to each element
            # overwrites (bit clear), later writes accumulate.
            def emit_gi(X, t, psg):
                xs = x_sb[X]
                row = xrow(X, t)
                for gt in range(6):
                    col = gt * 64 if gt < 4 else 256 + (gt - 4) * 64
                    for kc in range(2):
                        nc.tensor.matmul(
                            psg[:, col:col + 64],
                            wih_sb[X][:, kc, gt * 128:(gt + 1) * 128],
                            xs[:, kc, row, :],
                            start=(gt == 0 and kc == 0), stop=False,
                            skip_group_check=True)

            def emit_gh(X, t, psg):
                h = h_sb[X]
                # n-part (cols 384:512, first write overwrites), then rz accum
                for gt in (4, 5):
                    col = 384 + (gt - 4) * 64
                    for kc in range(2):
                        nc.tensor.matmul(
                            psg[:, col:col + 64],
                            whh_sb[X][:, kc, gt * 128:(gt + 1) * 128],
                            h[:, kc, :],
                            start=False, stop=False, skip_group_check=True)
                for gt in range(4):
                    col = gt * 64
                    for kc in range(2):
                        nc.tensor.matmul(
                            psg[:, col:col + 64],
                            whh_sb[X][:, kc, gt * 128:(gt + 1) * 128],
                            h[:, kc, :],
                            start=False, stop=(gt == 3 and kc == 1),
                            skip_group_check=True)

            def emit_conv(X, t, cy):
                # y[o,l] = sum_k sum_i cw[o,i,k] ctx[i,l+k-1], zero pad
                ctxrow = ctx_sb[X][:, :, xrow(X, t), :]  # [128,2,L]
                first = True
                for ot in range(2):
                    for kc in range(2):
                        for k in range(3):
                            if k == 0:
                                o_sl, i_sl = (1, L), (0, L - 1)
                            elif k == 1:
                                o_sl, i_sl = (0, L), (0, L)
                            else:
                                o_sl, i_sl = (0, L - 1), (1, L)
                            last = (not FOLD_CB and ot == 1 and kc == 1 and k == 2)
                            nc.tensor.matmul(
                                cy[:, ot, o_sl[0]:o_sl[1]],
                                cw_sb[X][:, kc, k, ot * 128:(ot + 1) * 128],
                                ctxrow[:, kc, i_sl[0]:i_sl[1]],
                                start=first, stop=last,
                                skip_group_check=True)
                            first = False
                    if FOLD_CB:
                        nc.tensor.matmul(
                            cy[:, ot, :], cb_sb[X][:, ot * 128:(ot + 1) * 128],
                            ones[:, 0:L], start=False, stop=(ot == 1),
                            skip_group_check=True)

            AFT, AFS = AF.Tanh, AF.Sigmoid
            AB = ABLATE
            psg_t = {}
            cy_t = {}
            for X in DIRS:
                psg_t[X] = ps.tile([128, 512], dt.float32, tag=f"g{X}", name=f"psg{X}")
                if 'gi' not in AB:
                    emit_gi(X, 0, psg_t[X])
            for t in range(nsteps):
                DIRS_T = DIRS
                for X in DIRS_T:
                    if 'gh' not in AB:
                        emit_gh(X, t, psg_t[X])
                rz = {}
                for X in DIRS_T:
                    rz[X] = work.tile([128, 256], dt.bfloat16, tag=f"rz{X}", name=f"rz{X}")
                    nc.scalar.activation(rz[X][:], psg_t[X][:, 0:256], AFS)
                t1, npre = {}, {}
                for X in DIRS_T:
                    if 'dve' in AB:
                        npre[X] = psg_t[X][:, 256:384].rearrange("p (a b) -> p a b", a=2)
                        continue
                    t1[X] = work.tile([128, 2, 64], dt.float32, tag=f"t1{X}", name=f"t1{X}")
                    nc.vector.tensor_mul(
                        t1[X][:], rz[X][:, 0:128].rearrange("p (a b) -> p a b", a=2),
                        psg_t[X][:, 384:512].rearrange("p (a b) -> p a b", a=2))
                    npre[X] = work.tile([128, 2, 64], dt.float32, tag=f"np{X}", name=f"np{X}")
                    nc.vector.tensor_add(
                        npre[X][:], t1[X][:],
                        psg_t[X][:, 256:384].rearrange("p (a b) -> p a b", a=2))
                nn = {}
                for X in DIRS_T:
                    nn[X] = work.tile([128, 2, 64], dt.bfloat16, tag=f"nn{X}", name=f"nn{X}")
                    nc.scalar.activation(nn[X][:], npre[X][:], AFT)
                for X in DIRS_T:
                    if 'dve' in AB:
                        nc.vector.tensor_copy(ctx_sb[X][:, :, xrow(X, t), :], nn[X][:])
                        continue
                    d1 = work.tile([128, 2, 64], dt.bfloat16, tag=f"d1{X}", name=f"d1{X}")
                    nc.vector.tensor_sub(d1[:], h_sb[X][:], nn[X][:])
                    d2 = work.tile([128, 2, 64], dt.bfloat16, tag=f"d2{X}", name=f"d2{X}")
                    nc.vector.tensor_mul(
                        d2[:], rz[X][:, 128:256].rearrange("p (a b) -> p a b", a=2), d1[:])
                    ctxrow = ctx_sb[X][:, :, xrow(X, t), :]
                    nc.vector.tensor_add(ctxrow, nn[X][:], d2[:])
                # next-step gi fills PE while elementwise completes
                if t + 1 < nsteps:
                    nxt = {}
                    for X in DIRS_T:
                        nxt[X] = ps.tile([128, 512], dt.float32, tag=f"g{X}", name=f"psg{X}")
                        if 'gi' not in AB:
                            emit_gi(X, t + 1, nxt[X])
                for X in DIRS_T:
                    if 'conv' in AB:
                        continue
                    cy_t[X] = ps.tile([128, 2, L], dt.float32, tag="convy", name=f"cy{X}")
                    emit_conv(X, t, cy_t[X])
                for X in DIRS_T:
                    src_ap = (psg_t[X][:, 0:128].rearrange("p (a b) -> p a b", a=2)
                              if 'conv' in AB else cy_t[X][:])
                    nc.scalar.activation(h_sb[X][:], src_ap, AFT)
                if t + 1 < nsteps:
                    psg_t = nxt

            if DEBUG_CTX:
                for X in DIRS:
                    nc.sync.dma_start(dbg[X][:], ctx_sb[X][:])
            # ---- combination matmul: out[o, h, w] ----
            for ot in range(2):
                if 'comb' in AB:
                    continue
                for ch in range(8):
                    h0 = ch * 8
                    pc = ps.tile([128, 512], dt.float32, tag="gd", name="pc")
                    first = True
                    for j, X in enumerate(DIRS):
                        for kc in range(2):
                            if X == 'r':
                                rhs = ctx_sb[X][:, kc, :, h0:h0 + 8].rearrange(
                                    "p w h -> p h w")
                            else:
                                rhs = ctx_sb[X][:, kc, h0:h0 + 8, :]
                            nc.tensor.matmul(
                                pc[:], wc_sb[:, j * 2 + kc, ot * 128:(ot + 1) * 128],
                                rhs, start=first, stop=False)
                            first = False
                    nc.tensor.matmul(pc[:], bc_sb[:, ot * 128:(ot + 1) * 128],
                                     ones[:], start=False, stop=True)
                    ob = obp.tile([128, 512], dt.float32, tag="ob")
                    if ch % 2 == 0:
                        nc.scalar.activation(ob[:], pc[:], AF.Copy)
                    else:
                        nc.vector.tensor_copy(ob[:], pc[:])
                    nc.sync.dma_start(
                        out_d[ot * 128:(ot + 1) * 128, h0:h0 + 8, :],
                        ob[:].rearrange("p (a b) -> p a b", a=8))

    nc.compile()
    return nc


def _prep_core_inputs(inputs, b):
    """Host-side layout prep for one batch sample (pure numpy, no device)."""
    def bf(a):
        return np.ascontiguousarray(a).astype(BF16)

    x = np.asarray(inputs['x'][b], np.float32)          # [C, H, W]
    m = {}
    m['xhw'] = bf(x.reshape(2, 128, H, W).transpose(1, 0, 2, 3))
    xw = x.transpose(0, 2, 1)                           # [C, W, H]
    m['xwh'] = bf(xw.reshape(2, 128, W, H).transpose(1, 0, 2, 3))
    for X in ('d', 'u', 'r'):
        wihm = np.asarray(inputs[f'wih_{X}'], np.float32)   # [768, 256]
        whhm = np.asarray(inputs[f'whh_{X}'], np.float32)
        cwm = np.asarray(inputs[f'cw_{X}'], np.float32)     # [256, 256, 3]
        cbm = np.asarray(inputs[f'cb_{X}'], np.float32)
        m[f'wihT_{X}'] = bf(wihm.T.reshape(2, 128, 768).transpose(1, 0, 2))
        m[f'whhT_{X}'] = bf(whhm.T.reshape(2, 128, 768).transpose(1, 0, 2))
        m[f'cwT_{X}'] = bf(cwm.transpose(1, 2, 0).reshape(2, 128, 3, 256)
                           .transpose(1, 0, 2, 3))
        m[f'cb_{X}'] = bf(cbm.reshape(1, 256))
    wc = np.asarray(inputs['w_comb'], np.float32)        # [256, 1024]
    m['wcombT'] = bf(wc[:, :768].T.reshape(3, 2, 128, 256)
                     .transpose(2, 0, 1, 3).reshape(128, 6, 256))
    m['bcomb'] = bf(np.asarray(inputs['b_comb'], np.float32).reshape(1, 256))
    return m


def _get_exec(nsteps=T):
    """Build + compile + wrap in a reusable jitted 8-core executable."""
    key = ('exec', nsteps, FOLD_CB)
    if key in _CACHE:
        return _CACHE[key]
    import jax
    jax.devices()  # initialize the PJRT plugin so platform "neuron" exists
    from jax.sharding import Mesh, PartitionSpec, NamedSharding
    from jax.experimental.shard_map import shard_map
    from concourse import bass2jax
    from jax.interpreters import mlir
    # bass2jax's import-time registration silently skips if the plugin
    # wasn't initialized yet; re-register unconditionally.
    mlir.register_lowering(bass2jax._bass_exec_p,
                           bass2jax._bass_exec_neuron_lowering,
                           platform="neuron")

    nc = _build(nsteps)
    _CACHE[('nc', nsteps)] = nc
    bass2jax.install_neuronx_cc_hook()

    in_names = ['xhw', 'xwh']
    for X in ('d', 'u', 'r'):
        in_names += [f'wihT_{X}', f'whhT_{X}', f'cwT_{X}', f'cb_{X}']
    in_names += ['wcombT', 'bcomb']
    out_names = ['out']
    out_avals = [jax.core.ShapedArray((256, H, W), np.float32)]
    if DEBUG_CTX:
        for X in ('d', 'u', 'r'):
            out_names.append(f'dbgctx_{X}')
            out_avals.append(jax.core.ShapedArray((128, 2, T, L), BF16))
    partition_name = nc.partition_id_tensor.name if nc.partition_id_tensor else None
    all_names = list(in_names) + out_names + ([partition_name] if partition_name else [])

    def _body(*args):
        operands = list(args)
        if partition_name is not None:
            operands.append(bass2jax.partition_id_tensor())
        outs = bass2jax._bass_exec_p.bind(
            *operands,
            out_avals=tuple(out_avals),
            in_names=tuple(all_names),
            out_names=tuple(out_names),
            lowering_input_output_aliases=(),
            sim_require_finite=True,
            sim_require_nnan=True,
            nc=nc,
        )
        return tuple(outs)

    devices = jax.devices()[:NCORES]
    mesh = Mesh(np.asarray(devices), ("core",))
    n_in = len(in_names) + len(out_names)
    sharded = jax.jit(
        shard_map(_body, mesh=mesh, in_specs=(PartitionSpec("core"),) * n_in,
                  out_specs=(PartitionSpec("core"),) * len(out_names),
                  check_rep=False),
        keep_unused=True,
    )
    sharding = NamedSharding(mesh, PartitionSpec("core"))
    _CACHE[key] = (sharded, in_names, sharding)
    return _CACHE[key]


def prep_concat_inputs(inputs, in_names):
    per_core = [_prep_core_inputs(inputs, b) for b in range(B)]
    cat = [np.concatenate([per_core[b][n] for b in range(B)], axis=0)
           for n in in_names]
    cat.append(np.zeros((B * 256, H, W), np.float32))   # donated out buffer
    if DEBUG_CTX:
        for _ in range(3):
            cat.append(np.zeros((B * 128, 2, T, L), BF16))
    return cat


def kernel(**inputs):
    import jax
    global FOLD_CB
    FOLD_CB = any(np.any(np.asarray(inputs[f'cb_{X}'])) for X in ('d', 'u', 'r'))
    sharded, in_names, sharding = _get_exec(T)
    cat = prep_concat_inputs(inputs, in_names)
    cat_d = [jax.device_put(a, sharding) for a in cat]
    (out,) = sharded(*cat_d)
    out = np.asarray(out).reshape(B, 256, H, W)
    return out


if __name__ == '__main__':
    # quick smoke test with random inputs
    rng = np.random.default_rng(0)
    ins = {'x': rng.standard_normal((B, C, H, W), np.float32) * 0.1}
    for X in ('d', 'u', 'l', 'r'):
        ins[f'wih_{X}'] = rng.standard_normal((768, 256), np.float32) / 16
        ins[f'whh_{X}'] = rng.standard_normal((768, 256), np.float32) / 16
        ins[f'cw_{X}'] = rng.standard_normal((256, 256, 3), np.float32) / 16
        ins[f'cb_{X}'] = np.zeros(256, np.float32)
    ins['w_comb'] = rng.standard_normal((256, 1024), np.float32) / 32
    ins['b_comb'] = np.zeros(256, np.float32)
    o = kernel(**ins)
    print("out", o.shape, o.dtype, float(np.abs(o).max()))


# revision 34
# speedup vs baseline: 1.5503x; 1.0452x over previous
"""Trainium2 Bass kernel for nn_CSRN_76922864272117.

Quad-directional conv-GRU spatial RNN (CSRN). B=8,C=256,H=W=64.
Reference computes 4 directional scans but the 'left' scan result is dead
(overwritten) and ctx_right is zeros, so only 3 scans (down, up, right)
and the first 3 C-blocks of w_comb contribute to the output.

Sharding: data-parallel over batch, 1 sample per NeuronCore (8 cores).
Per core: 3 independent 64-step recurrent chains, gates computed in
[g-tile, l] layout (128 partitions), gi+gh fused in PSUM, conv1d as 3
shifted matmuls (+K=1 bias matmul), sigmoid/tanh on ScalarE from PSUM,
final 768->256 combination matmul tail. All matmul inputs bf16 (fp32
PSUM accumulate): measured rel_err vs fp32 reference ~5e-3.

Host-side prep (outside the NEFF): transpose/cast weights & x into the
SBUF-friendly layouts; gather = np.stack of per-core outputs.

Schedule: phase-major emission per step (gh x3 dirs, elementwise x3,
next-step gi x3 as PE fill, conv x3, conv-tanh x3) pipelines the three
chains at a flat ~4.5us/step; input DMAs are chunked and ordered by
first use so the scan starts ~5us in. Conv bias matmuls are elided when
cb==0 (always true per the spec; FOLD_CB rebuild handles nonzero).
"""
import sys
if '/opt/trn_rl_repo' not in sys.path:
    sys.path.insert(0, '/opt/trn_rl_repo')

import numpy as np
import ml_dtypes

B, C, H, W = 8, 256, 64, 64
T = 64          # scan steps (H or W)
L = 64          # line length per step
NCORES = 8
BF16 = ml_dtypes.bfloat16

_CACHE = {}
DEBUG_CTX = False
FOLD_CB = False   # emit K=1 conv-bias matmuls (needed only if cb != 0)
ABLATE = frozenset()   # timing-only ablations: gi, gh, conv, dve, comb


def _build(nsteps):
    import concourse.bass as bass
    import concourse.tile as tile
    import concourse.mybir as mybir
    from concourse import bacc

    dt = mybir.dt
    AF = mybir.ActivationFunctionType
    nc = bacc.Bacc(None, target_bir_lowering=False)

    DIRS = ('d', 'u', 'r')
    # ---- DRAM parameters (host-prepped layouts) ----
    xhw = nc.dram_tensor("xhw", [128, 2, H, W], dt.bfloat16, kind="ExternalInput")
    xwh = nc.dram_tensor("xwh", [128, 2, W, H], dt.bfloat16, kind="ExternalInput")
    wih, whh, cwT, cb = {}, {}, {}, {}
    for X in DIRS:
        wih[X] = nc.dram_tensor(f"wihT_{X}", [128, 2, 768], dt.bfloat16, kind="ExternalInput")
        whh[X] = nc.dram_tensor(f"whhT_{X}", [128, 2, 768], dt.bfloat16, kind="ExternalInput")
        cwT[X] = nc.dram_tensor(f"cwT_{X}", [128, 2, 3, 256], dt.bfloat16, kind="ExternalInput")
        cb[X] = nc.dram_tensor(f"cb_{X}", [1, 256], dt.bfloat16, kind="ExternalInput")
    wcombT = nc.dram_tensor("wcombT", [128, 6, 256], dt.bfloat16, kind="ExternalInput")
    bcomb = nc.dram_tensor("bcomb", [1, 256], dt.bfloat16, kind="ExternalInput")
    out_d = nc.dram_tensor("out", [256, H, W], dt.float32, kind="ExternalOutput")
    dbg = {}
    if DEBUG_CTX:
        for X in DIRS:
            dbg[X] = nc.dram_tensor(f"dbgctx_{X}", [128, 2, T, L], dt.bfloat16,
                                    kind="ExternalOutput")

    with tile.TileContext(nc) as tc:
        with (
            tc.tile_pool(name="big", bufs=1) as big,
            tc.tile_pool(name="work", bufs=3) as work,
            tc.tile_pool(name="ob", bufs=3) as obp,
            tc.tile_pool(name="ps", bufs=2, space="PSUM") as ps,
        ):
            # ---- persistent SBUF tensors ----
            x_sb = {}
            x_sb['d'] = big.tile([128, 2, H, W], dt.bfloat16, tag="xhw", name="xhw_sb")
            x_sb['u'] = x_sb['d']
            x_sb['r'] = big.tile([128, 2, W, H], dt.bfloat16, tag="xwh", name="xwh_sb")
            wih_sb, whh_sb, cw_sb, cb_sb, h_sb, ctx_sb = {}, {}, {}, {}, {}, {}
            for X in DIRS:
                wih_sb[X] = big.tile([128, 2, 768], dt.bfloat16, tag=f"wih{X}", name=f"wih{X}_sb")
                whh_sb[X] = big.tile([128, 2, 768], dt.bfloat16, tag=f"whh{X}", name=f"whh{X}_sb")
                cw_sb[X] = big.tile([128, 2, 3, 256], dt.bfloat16, tag=f"cw{X}", name=f"cw{X}_sb")
                cb_sb[X] = big.tile([1, 256], dt.bfloat16, tag=f"cb{X}", name=f"cb{X}_sb")
                h_sb[X] = big.tile([128, 2, L], dt.bfloat16, tag=f"h{X}", name=f"h{X}_sb")
                nc.vector.memset(h_sb[X][:], 0.0)
                ctx_sb[X] = big.tile([128, 2, T, L], dt.bfloat16, tag=f"ctx{X}", name=f"ctx{X}_sb")
                if nsteps < T:
                    nc.vector.memset(ctx_sb[X][:], 0.0)  # debug-only builds
            # DMA order = first-use order: per-dir gi weights + the x rows
            # each scan touches first, then recurrence weights, then the x
            # remainders in consumption order (d forward, u/r backward).
            nc.sync.dma_start(wih_sb['d'][:], wih['d'][:])
            nc.sync.dma_start(x_sb['d'][:, :, 0:4, :], xhw[:, :, 0:4, :])
            nc.sync.dma_start(wih_sb['u'][:], wih['u'][:])
            nc.sync.dma_start(x_sb['d'][:, :, 60:64, :], xhw[:, :, 60:64, :])
            nc.sync.dma_start(wih_sb['r'][:], wih['r'][:])
            nc.sync.dma_start(x_sb['r'][:, :, 60:64, :], xwh[:, :, 60:64, :])
            for X in DIRS:
                nc.sync.dma_start(whh_sb[X][:], whh[X][:])
            for X in DIRS:
                nc.sync.dma_start(cw_sb[X][:], cwT[X][:])
                nc.sync.dma_start(cb_sb[X][:], cb[X][:])
            nc.sync.dma_start(x_sb['d'][:, :, 4:12, :], xhw[:, :, 4:12, :])
            nc.sync.dma_start(x_sb['d'][:, :, 52:60, :], xhw[:, :, 52:60, :])
            nc.sync.dma_start(x_sb['r'][:, :, 52:60, :], xwh[:, :, 52:60, :])
            nc.sync.dma_start(x_sb['d'][:, :, 36:52, :], xhw[:, :, 36:52, :])
            nc.sync.dma_start(x_sb['r'][:, :, 36:52, :], xwh[:, :, 36:52, :])
            nc.sync.dma_start(x_sb['d'][:, :, 12:36, :], xhw[:, :, 12:36, :])
            nc.sync.dma_start(x_sb['r'][:, :, 0:36, :], xwh[:, :, 0:36, :])
            wc_sb = big.tile([128, 6, 256], dt.bfloat16, tag="wcomb")
            bc_sb = big.tile([1, 256], dt.bfloat16, tag="bcomb")
            nc.sync.dma_start(wc_sb[:], wcombT[:])
            nc.sync.dma_start(bc_sb[:], bcomb[:])
            ones = big.tile([1, 512], dt.bfloat16, tag="ones")
            nc.vector.memset(ones[:], 1.0)

            def xrow(X, t):
                return t if X == 'd' else (T - 1 - t)

            # NOTE: start=True clears has_written for the WHOLE psum bank, so
            # each 512-wide psg tile gets exactly one start (first matmul of
            # the step) and one stop (last); first write to each element
            # overwrites (bit clear), later writes accumulate.
            def emit_gi(X, t, psg):
                xs = x_sb[X]
                row = xrow(X, t)
                for gt in range(6):
                    col = gt * 64 if gt < 4 else 256 + (gt - 4) * 64
                    for kc in range(2):
                        nc.tensor.matmul(
                            psg[:, col:col + 64],
                            wih_sb[X][:, kc, gt * 128:(gt + 1) * 128],
                            xs[:, kc, row, :],
                            start=(gt == 0 and kc == 0), stop=False,
                            skip_group_check=True)

            def emit_gh(X, t, psg):
                h = h_sb[X]
                # n-part (cols 384:512, first write overwrites), then rz accum
                for gt in (4, 5):
                    col = 384 + (gt - 4) * 64
                    for kc in range(2):
                        nc.tensor.matmul(
                            psg[:, col:col + 64],
                            whh_sb[X][:, kc, gt * 128:(gt + 1) * 128],
                            h[:, kc, :],
                            start=False, stop=False, skip_group_check=True)
                for gt in range(4):
                    col = gt * 64
                    for kc in range(2):
                        nc.tensor.matmul(
                            psg[:, col:col + 64],
                            whh_sb[X][:, kc, gt * 128:(gt + 1) * 128],
                            h[:, kc, :],
                            start=False, stop=(gt == 3 and kc == 1),
                            skip_group_check=True)

            def emit_conv(X, t, cy):
                # y[o,l] = sum_k sum_i cw[o,i,k] ctx[i,l+k-1], zero pad
                ctxrow = ctx_sb[X][:, :, xrow(X, t), :]  # [128,2,L]
                first = True
                for ot in range(2):
                    for kc in range(2):
                        for k in range(3):
                            if k == 0:
                                o_sl, i_sl = (1, L), (0, L - 1)
                            elif k == 1:
                                o_sl, i_sl = (0, L), (0, L)
                            else:
                                o_sl, i_sl = (0, L - 1), (1, L)
                            last = (not FOLD_CB and ot == 1 and kc == 1 and k == 2)
                            nc.tensor.matmul(
                                cy[:, ot, o_sl[0]:o_sl[1]],
                                cw_sb[X][:, kc, k, ot * 128:(ot + 1) * 128],
                                ctxrow[:, kc, i_sl[0]:i_sl[1]],
                                start=first, stop=last,
                                skip_group_check=True)
                            first = False
                    if FOLD_CB:
                        nc.tensor.matmul(
                            cy[:, ot, :], cb_sb[X][:, ot * 128:(ot + 1) * 128],
                            ones[:, 0:L], start=False, stop=(ot == 1),
                            skip_group_check=True)

            AFT, AFS = AF.Tanh, AF.Sigmoid
            AB = ABLATE
            psg_t = {}
            cy_t = {}
            for X in DIRS:
                psg_t[X] = ps.tile([128, 512], dt.float32, tag=f"g{X}", name=f"psg{X}")
                if 'gi' not in AB:
                    emit_gi(X, 0, psg_t[X])
            for t in range(nsteps):
                DIRS_T = DIRS
                for X in DIRS_T:
                    if 'gh' not in AB:
                        emit_gh(X, t, psg_t[X])
                rz = {}
                for X in DIRS_T:
                    rz[X] = work.tile([128, 256], dt.bfloat16, tag=f"rz{X}", name=f"rz{X}")
                    nc.scalar.activation(rz[X][:], psg_t[X][:, 0:256], AFS)
                t1, npre = {}, {}
                for X in DIRS_T:
                    if 'dve' in AB:
                        npre[X] = psg_t[X][:, 256:384].rearrange("p (a b) -> p a b", a=2)
                        continue
                    t1[X] = work.tile([128, 2, 64], dt.float32, tag=f"t1{X}", name=f"t1{X}")
                    nc.vector.tensor_mul(
                        t1[X][:], rz[X][:, 0:128].rearrange("p (a b) -> p a b", a=2),
                        psg_t[X][:, 384:512].rearrange("p (a b) -> p a b", a=2))
                    npre[X] = work.tile([128, 2, 64], dt.float32, tag=f"np{X}", name=f"np{X}")
                    nc.vector.tensor_add(
                        npre[X][:], t1[X][:],
                        psg_t[X][:, 256:384].rearrange("p (a b) -> p a b", a=2))
                nn = {}
                for X in DIRS_T:
                    nn[X] = work.tile([128, 2, 64], dt.bfloat16, tag=f"nn{X}", name=f"nn{X}")
                    nc.scalar.activation(nn[X][:], npre[X][:], AFT)
                for X in DIRS_T:
                    if 'dve' in AB:
                        nc.vector.tensor_copy(ctx_sb[X][:, :, xrow(X, t), :], nn[X][:])
                        continue
                    d1 = work.tile([128, 2, 64], dt.bfloat16, tag=f"d1{X}", name=f"d1{X}")
                    nc.vector.tensor_sub(d1[:], h_sb[X][:], nn[X][:])
                    d2 = work.tile([128, 2, 64], dt.bfloat16, tag=f"d2{X}", name=f"d2{X}")
                    nc.vector.tensor_mul(
                        d2[:], rz[X][:, 128:256].rearrange("p (a b) -> p a b", a=2), d1[:])
                    ctxrow = ctx_sb[X][:, :, xrow(X, t), :]
                    nc.vector.tensor_add(ctxrow, nn[X][:], d2[:])
                # next-step gi fills PE while elementwise completes
                if t + 1 < nsteps:
                    nxt = {}
                    for X in DIRS_T:
                        nxt[X] = ps.tile([128, 512], dt.float32, tag=f"g{X}", name=f"psg{X}")
                        if 'gi' not in AB:
                            emit_gi(X, t + 1, nxt[X])
                for X in DIRS_T:
                    if 'conv' in AB:
                        continue
                    cy_t[X] = ps.tile([128, 2, L], dt.float32, tag="convy", name=f"cy{X}")
                    emit_conv(X, t, cy_t[X])
                for X in DIRS_T:
                    src_ap = (psg_t[X][:, 0:128].rearrange("p (a b) -> p a b", a=2)
                              if 'conv' in AB else cy_t[X][:])
                    nc.scalar.activation(h_sb[X][:], src_ap, AFT)
                if t + 1 < nsteps:
                    psg_t = nxt

            if DEBUG_CTX:
                for X in DIRS:
                    nc.sync.dma_start(dbg[X][:], ctx_sb[X][:])
            # ---- combination matmul: out[o, h, w] ----
            for ot in range(2):
                if 'comb' in AB:
                    continue
                for ch in range(8):
                    h0 = ch * 8
                    pc = ps.tile([128, 512], dt.float32, tag="gd", name="pc")
                    first = True
                    for j, X in enumerate(DIRS):
                        for kc in range(2):
                            if X == 'r':
                                rhs = ctx_sb[X][:, kc, :, h0:h0 + 8].rearrange(
                                    "p w h -> p h w")
                            else:
                                rhs = ctx_sb[X][:, kc, h0:h0 + 8, :]
                            nc.tensor.matmul(
                                pc[:], wc_sb[:, j * 2 + kc, ot * 128:(ot + 1) * 128],
                                rhs, start=first, stop=False)
                            first = False
                    nc.tensor.matmul(pc[:], bc_sb[:, ot * 128:(ot + 1) * 128],
                                     ones[:], start=False, stop=True)
                    ob = obp.tile([128, 512], dt.float32, tag="ob")
                    if ch % 2 == 0:
                        nc.scalar.activation(ob[:], pc[:], AF.Copy)
                    else:
                        nc.vector.tensor_copy(ob[:], pc[:])
                    nc.sync.dma_start(
                        out_d[ot * 128:(ot + 1) * 128, h0:h0 + 8, :],
                        ob[:].rearrange("p (a b) -> p a b", a=8))

    nc.compile()
    return nc


def _prep_core_inputs(inputs, b):
    """Host-side layout prep for one batch sample (pure numpy, no device)."""
    def bf(a):
        return np.ascontiguousarray(a).astype(BF16)

    x = np.asarray(inputs['x'][b], np.float32)          # [C, H, W]
    m = {}
    m['xhw'] = bf(x.reshape(2, 128, H, W).transpose(1, 0, 2, 3))
    xw = x.transpose(0, 2, 1)                           # [C, W, H]
    m['xwh'] = bf(xw.reshape(2, 128, W, H).transpose(1, 0, 2, 3))
    for X in ('d', 'u', 'r'):
        wihm = np.asarray(inputs[f'wih_{X}'], np.float32)   # [768, 256]
        whhm = np.asarray(inputs[f'whh_{X}'], np.float32)
        cwm = np.asarray(inputs[f'cw_{X}'], np.float32)     # [256, 256, 3]
        cbm = np.asarray(inputs[f'cb_{X}'], np.float32)
        m[f'wihT_{X}'] = bf(wihm.T.reshape(2, 128, 768).transpose(1, 0, 2))
        m[f'whhT_{X}'] = bf(whhm.T.reshape(2, 128, 768).transpose(1, 0, 2))
        m[f'cwT_{X}'] = bf(cwm.transpose(1, 2, 0).reshape(2, 128, 3, 256)
                           .transpose(1, 0, 2, 3))
        m[f'cb_{X}'] = bf(cbm.reshape(1, 256))
    wc = np.asarray(inputs['w_comb'], np.float32)        # [256, 1024]
    m['wcombT'] = bf(wc[:, :768].T.reshape(3, 2, 128, 256)
                     .transpose(2, 0, 1, 3).reshape(128, 6, 256))
    m['bcomb'] = bf(np.asarray(inputs['b_comb'], np.float32).reshape(1, 256))
    return m


def _get_exec(nsteps=T):
    """Build + compile + wrap in a reusable jitted 8-core executable."""
    key = ('exec', nsteps, FOLD_CB)
    if key in _CACHE:
        return _CACHE[key]
    import jax
    jax.devices()  # initialize the PJRT plugin so platform "neuron" exists
    from jax.sharding import Mesh, PartitionSpec, NamedSharding
    from jax.experimental.shard_map import shard_map
    from concourse import bass2jax
    from jax.interpreters import mlir
    # bass2jax's import-time registration silently skips if the plugin
    # wasn't initialized yet; re-register unconditionally.
    mlir.register_lowering(bass2jax._bass_exec_p,
                           bass2jax._bass_exec_neuron_lowering,
                           platform="neuron")

    nc = _build(nsteps)
    _CACHE[('nc', nsteps)] = nc
    bass2jax.install_neuronx_cc_hook()

    in_names = ['xhw', 'xwh']
    for X in ('d', 'u', 'r'):
        in_names += [f'wihT_{X}', f'whhT_{X}', f'cwT_{X}', f'cb_{X}']
    in_names += ['wcombT', 'bcomb']
    out_names = ['out']
    out_avals = [jax.core.ShapedArray((256, H, W), np.float32)]
    if DEBUG_CTX:
        for X in ('d', 'u', 'r'):
            out_names.append(f'dbgctx_{X}')
            out_avals.append(jax.core.ShapedArray((128, 2, T, L), BF16))
    partition_name = nc.partition_id_tensor.name if nc.partition_id_tensor else None
    all_names = list(in_names) + out_names + ([partition_name] if partition_name else [])

    def _body(*args):
        operands = list(args)
        if partition_name is not None:
            operands.append(bass2jax.partition_id_tensor())
        outs = bass2jax._bass_exec_p.bind(
            *operands,
            out_avals=tuple(out_avals),
            in_names=tuple(all_names),
            out_names=tuple(out_names),
            lowering_input_output_aliases=(),
            sim_require_finite=True,
            sim_require_nnan=True,
            nc=nc,
        )
        return tuple(outs)

    devices = jax.devices()[:NCORES]
    mesh = Mesh(np.asarray(devices), ("core",))
    n_in = len(in_names) + len(out_names)
    sharded = jax.jit(
        shard_map(_body, mesh=mesh, in_specs=(PartitionSpec("core"),) * n_in,
                  out_specs=(PartitionSpec("core"),) * len(out_names),
                  check_rep=False),
        keep_unused=True,
    )
    sharding = NamedSharding(mesh, PartitionSpec("core"))
    _CACHE[key] = (sharded, in_names, sharding)
    return _CACHE[key]


def prep_concat_inputs(inputs, in_names):
    per_core = [_prep_core_inputs(inputs, b) for b in range(B)]
    cat = [np.concatenate([per_core[b][n] for b in range(B)], axis=0)
           for n in in_names]
    cat.append(np.zeros((B * 256, H, W), np.float32))   # donated out buffer
    if DEBUG_CTX:
        for _ in range(3):
            cat.append(np.zeros((B * 128, 2, T, L), BF16))
    return cat


def kernel(**inputs):
    import jax
    global FOLD_CB
    FOLD_CB = any(np.any(np.asarray(inputs[f'cb_{X}'])) for X in ('d', 'u', 'r'))
    sharded, in_names, sharding = _get_exec(T)
    cat = prep_concat_inputs(inputs, in_names)
    cat_d = [jax.device_put(a, sharding) for a in cat]
    (out,) = sharded(*cat_d)
    out = np.asarray(out).reshape(B, 256, H, W)
    return out


if __name__ == '__main__':
    # quick smoke test with random inputs
    rng = np.random.default_rng(0)
    ins = {'x': rng.standard_normal((B, C, H, W), np.float32) * 0.1}
    for X in ('d', 'u', 'l', 'r'):
        ins[f'wih_{X}'] = rng.standard_normal((768, 256), np.float32) / 16
        ins[f'whh_{X}'] = rng.standard_normal((768, 256), np.float32) / 16
        ins[f'cw_{X}'] = rng.standard_normal((256, 256, 3), np.float32) / 16
        ins[f'cb_{X}'] = np.zeros(256, np.float32)
    ins['w_comb'] = rng.standard_normal((256, 1024), np.float32) / 32
    ins['b_comb'] = np.zeros(256, np.float32)
    o = kernel(**ins)
    print("out", o.shape, o.dtype, float(np.abs(o).max()))
